# revision 7
# baseline (speedup 1.0000x reference)
"""CLAM-SB attention-MIL forward pass on 8 Trainium2 NeuronCores.

Strategy (sharding_hint: shard instance dim N across devices):
  - N=50000 rows of h are split 8 ways (6250/core, zero-padded to 6272=14*448).
  - The host pre-transposes each shard to hT [1024, 6272] so the contraction
    dim lands on SBUF partitions with no on-device transposes.
  - Each core streams 14 column-slices of 448 instances:
      h512T = relu(W_fc^T @ hT)            (4x 128-partition chunks, resident per slice)
      aT    = tanh(Wa^T @ h512T), bT = sigmoid(Wb^T @ h512T)
      s     = Wc^T @ (aT*bT)  -- computed with Wc replicated across 128 output
              partitions, so exp(s) is already broadcast for the pooling step
      Mpart += sum_n exp(s_n) * h512T[:, n]  (fused mul+reduce on DVE)
    Scores are tiny (~|s|<0.5) so unnormalized exp() is safe: no global max/
    normalization dependency on device; the host divides by Z = sum exp(s).
  - Outputs per core: raw scores (the A_raw shard) + the [512] pooled partial.
  - Host: merge scores -> global softmax norm, global top/bottom-64 ("all-gather
    k candidates and reduce"), recompute the 128 selected h512 rows in f32
    (tiny), instance loss + bag classifier heads.
"""
import os
import sys

import numpy as np

sys.path.insert(0, "/opt/trn_rl_repo")

N, L, D1, D2, K, C, T = 50000, 1024, 512, 256, 64, 2, 49
NCORES = 8
NV = N // NCORES            # 6250 valid instances per core
NS = 448                    # slice width (<=512 psum f32; >=256 for f32r full rate)
NSLICES = 14
NP = NS * NSLICES           # 6272 padded
KL = L // 128               # 8 k-chunks over the fc contraction
KD = D1 // 128              # 4 chunks over d1
KE = D2 // 128              # 2 chunks over d2
VALID_LAST = NV - (NSLICES - 1) * NS   # 426 valid cols in the final slice

# matmul dtype: "fp16" (default) or "bf16"
MM_MODE = os.environ.get("KERNEL_MM_MODE", "fp16")

_CACHE = {}


def _build_program():
    import concourse.bass as bass
    import concourse.bacc as bacc
    import concourse.mybir as mybir
    import concourse.tile as tile
    from concourse.mybir import dt

    f32 = dt.float32
    # fp16: full PE rate (1 cyc/row), legal on ACT/DVE (unlike float32r whose
    # producer-dtype rule forbids DVE/ACT in the chain), ~10-bit mantissa.
    DT = dt.bfloat16 if MM_MODE == "bf16" else dt.float16

    def mmap(ap):
        return ap

    AF = mybir.ActivationFunctionType
    Alu = mybir.AluOpType
    Axis = mybir.AxisListType

    nc = bacc.Bacc("TRN2", target_bir_lowering=False, debug=False)

    hT_d = nc.dram_tensor("hT", [L, NP], DT, kind="ExternalInput").ap()
    wfc_d = nc.dram_tensor("Wfc", [L, D1], DT, kind="ExternalInput").ap()
    wa_d = nc.dram_tensor("Wa", [D1, D2], DT, kind="ExternalInput").ap()
    wb_d = nc.dram_tensor("Wb", [D1, D2], DT, kind="ExternalInput").ap()
    wc_d = nc.dram_tensor("Wc", [D2, 1], DT, kind="ExternalInput").ap()
    bfc_d = nc.dram_tensor("bfc", [D1, 1], f32, kind="ExternalInput").ap()
    ba_d = nc.dram_tensor("ba", [D2, 1], f32, kind="ExternalInput").ap()
    bb_d = nc.dram_tensor("bb", [D2, 1], f32, kind="ExternalInput").ap()
    bc_d = nc.dram_tensor("bc", [1, 1], f32, kind="ExternalInput").ap()
    s_out_d = nc.dram_tensor("s_out", [NSLICES, NS], f32, kind="ExternalOutput").ap()
    m_out_d = nc.dram_tensor("M_out", [KD, 128], f32, kind="ExternalOutput").ap()

    with tile.TileContext(nc) as tc:
        with (
            tc.tile_pool(name="consts", bufs=1) as consts,
            tc.tile_pool(name="io", bufs=2 * KL) as io,
            tc.tile_pool(name="h512p", bufs=2 * KD) as h512p,
            tc.tile_pool(name="abp", bufs=4) as abp,
            tc.tile_pool(name="wp", bufs=2) as wp,
            tc.tile_pool(name="accp", bufs=1) as accp,
            tc.tile_pool(name="psfc", bufs=3, space="PSUM") as psfc,
            tc.tile_pool(name="psab", bufs=4, space="PSUM") as psab,
            tc.tile_pool(name="pss", bufs=1, space="PSUM") as pss,
        ):
            # ---- constants ----
            wfc = []
            for k in range(KL):
                wfck = consts.tile([128, D1], DT, name=f"wfc{k}")
                nc.sync.dma_start(out=wfck, in_=wfc_d[k * 128:(k + 1) * 128, :])
                wfc.append(wfck)
            wa, wb = [], []
            for d in range(KD):
                wad = consts.tile([128, D2], DT, name=f"wa{d}")
                nc.sync.dma_start(out=wad, in_=wa_d[d * 128:(d + 1) * 128, :])
                wa.append(wad)
                wbd = consts.tile([128, D2], DT, name=f"wb{d}")
                nc.sync.dma_start(out=wbd, in_=wb_d[d * 128:(d + 1) * 128, :])
                wb.append(wbd)
            wc_col, wc_rep = [], []
            for e in range(KE):
                wce = consts.tile([128, 1], DT, name=f"wc{e}")
                nc.sync.dma_start(out=wce, in_=wc_d[e * 128:(e + 1) * 128, :])
                wc_col.append(wce)
                wcr = consts.tile([128, 128], DT, name=f"wcrep{e}")
                nc.vector.tensor_copy(wcr[:, :], wce[:, 0:1].broadcast_to((128, 128)))
                wc_rep.append(wcr)
            bfc_c = []
            for d in range(KD):
                bfcd = consts.tile([128, 1], f32, name=f"bfc{d}")
                nc.sync.dma_start(out=bfcd, in_=bfc_d[d * 128:(d + 1) * 128, :])
                bfc_c.append(bfcd)
            ba_c, bb_c = [], []
            for e in range(KE):
                bae = consts.tile([128, 1], f32, name=f"ba{e}")
                nc.sync.dma_start(out=bae, in_=ba_d[e * 128:(e + 1) * 128, :])
                ba_c.append(bae)
                bbe = consts.tile([128, 1], f32, name=f"bb{e}")
                nc.sync.dma_start(out=bbe, in_=bb_d[e * 128:(e + 1) * 128, :])
                bb_c.append(bbe)
            bc_sb = consts.tile([1, 1], f32, name="bc_sb")
            nc.sync.dma_start(out=bc_sb, in_=bc_d[:, :])
            bc_rep = consts.tile([128, 1], f32, name="bc_rep")
            nc.gpsimd.partition_broadcast(bc_rep[:, :], bc_sb[:, :])

            macc = []
            for d in range(KD):
                md = accp.tile([128, NSLICES], f32, name=f"macc{d}")
                macc.append(md)

            # ---- streaming over instance slices ----
            for j in range(NSLICES):
                ht = []
                for k in range(KL):
                    htk = io.tile([128, NS], DT, tag="ht")
                    nc.sync.dma_start(
                        out=htk, in_=hT_d[k * 128:(k + 1) * 128, j * NS:(j + 1) * NS]
                    )
                    ht.append(htk)

                h512 = []
                for d in range(KD):
                    ps = psfc.tile([128, NS], f32, tag="psfc")
                    for k in range(KL):
                        nc.tensor.matmul(
                            ps[:, :],
                            mmap(wfc[k][:, d * 128:(d + 1) * 128]),
                            mmap(ht[k][:, :]),
                            start=(k == 0),
                            stop=(k == KL - 1),
                        )
                    hd = h512p.tile([128, NS], DT, tag="h512")
                    nc.scalar.activation(hd[:, :], ps[:, :], AF.Relu, bias=bfc_c[d][:, :])
                    h512.append(hd)

                ab = []
                for e in range(KE):
                    psa = psab.tile([128, NS], f32, tag="psab")
                    for d in range(KD):
                        nc.tensor.matmul(
                            psa[:, :],
                            mmap(wa[d][:, e * 128:(e + 1) * 128]),
                            mmap(h512[d][:, :]),
                            start=(d == 0),
                            stop=(d == KD - 1),
                        )
                    at = abp.tile([128, NS], DT, tag="at")
                    nc.scalar.activation(at[:, :], psa[:, :], AF.Tanh, bias=ba_c[e][:, :])

                    psb = psab.tile([128, NS], f32, tag="psab")
                    for d in range(KD):
                        nc.tensor.matmul(
                            psb[:, :],
                            mmap(wb[d][:, e * 128:(e + 1) * 128]),
                            mmap(h512[d][:, :]),
                            start=(d == 0),
                            stop=(d == KD - 1),
                        )
                    bt = abp.tile([128, NS], DT, tag="bt")
                    nc.scalar.activation(bt[:, :], psb[:, :], AF.Sigmoid, bias=bb_c[e][:, :])

                    abe = abp.tile([128, NS], DT, tag="ab")
                    nc.vector.tensor_mul(abe[:, :], at[:, :], bt[:, :])
                    ab.append(abe)

                # s replicated across all 128 partitions
                ps_s = pss.tile([128, NS], f32, tag="pss")
                for e in range(KE):
                    nc.tensor.matmul(
                        ps_s[:, :],
                        mmap(wc_rep[e][:, :]),
                        mmap(ab[e][:, :]),
                        start=(e == 0),
                        stop=(e == KE - 1),
                    )
                srow = wp.tile([1, NS], f32, tag="srow")
                nc.scalar.activation(srow[:, :], ps_s[0:1, :], AF.Identity, bias=bc_sb[:, :])
                nc.sync.dma_start(out=s_out_d[j, :], in_=srow[:, :])

                wbc = wp.tile([128, NS], DT, tag="wbc")
                nc.scalar.activation(wbc[:, :], ps_s[:, :], AF.Exp, bias=bc_rep[:, :])
                if j == NSLICES - 1:
                    nc.vector.memset(wbc[:, VALID_LAST:], 0.0)

                for d in range(KD):
                    # fused multiply + free-dim-sum on DVE (TensorTensorReduce
                    # crashes this runtime; STT's accum_out does the same job)
                    prod = wp.tile([128, NS], f32, tag="prod")
                    nc.vector.scalar_tensor_tensor(
                        out=prod[:, :],
                        in0=h512[d][:, :],
                        scalar=1.0,
                        in1=wbc[:, :],
                        op0=Alu.mult,
                        op1=Alu.mult,
                        accum_out=macc[d][:, j:j + 1],
                    )

            # ---- final M reduction ----
            for d in range(KD):
                mfin = wp.tile([128, 1], f32, tag="mfin")
                nc.vector.tensor_reduce(mfin[:, :], macc[d][:, :], axis=Axis.X, op=Alu.add)
                nc.sync.dma_start(out=m_out_d[d, :], in_=mfin[:, :])

    nc.compile()
    return nc


def _get_program():
    if "nc" not in _CACHE:
        _CACHE["nc"] = _build_program()
    return _CACHE["nc"]


def _np_dt():
    if MM_MODE == "bf16":
        import ml_dtypes
        return ml_dtypes.bfloat16
    return np.float16


def _prepare_in_maps(h, W_fc, Wa, Wb, Wc, b_fc, ba, bb, bc):
    ndt = _np_dt()
    base = {
        "Wfc": np.ascontiguousarray(W_fc, dtype=ndt),
        "Wa": np.ascontiguousarray(Wa, dtype=ndt),
        "Wb": np.ascontiguousarray(Wb, dtype=ndt),
        "Wc": np.ascontiguousarray(Wc, dtype=ndt).reshape(D2, 1),
        "bfc": np.ascontiguousarray(b_fc, dtype=np.float32).reshape(D1, 1),
        "ba": np.ascontiguousarray(ba, dtype=np.float32).reshape(D2, 1),
        "bb": np.ascontiguousarray(bb, dtype=np.float32).reshape(D2, 1),
        "bc": np.ascontiguousarray(bc, dtype=np.float32).reshape(1, 1),
    }
    in_maps = []
    hT = np.ascontiguousarray(np.asarray(h, dtype=np.float32).T)   # [L, N]
    for c in range(NCORES):
        hT_pad = np.zeros((L, NP), dtype=ndt)
        hT_pad[:, :NV] = hT[:, c * NV:(c + 1) * NV]
        m = dict(base)
        m["hT"] = hT_pad
        in_maps.append(m)
    return in_maps


def _run_device(h, W_fc, Wa, Wb, Wc, b_fc, ba, bb, bc, trace=False):
    from concourse.bass_utils import run_bass_kernel_spmd

    nc = _get_program()
    in_maps = _prepare_in_maps(h, W_fc, Wa, Wb, Wc, b_fc, ba, bb, bc)
    res = run_bass_kernel_spmd(nc, in_maps, list(range(NCORES)), trace=trace)
    s_full = np.zeros(N, np.float32)
    M_unnorm = np.zeros(D1, np.float64)
    for c in range(NCORES):
        out = res.results[c]
        s_full[c * NV:(c + 1) * NV] = out["s_out"].reshape(-1)[:NV]
        M_unnorm += out["M_out"].reshape(-1).astype(np.float64)
    return s_full, M_unnorm, res


def _finish_host(s_full, M_unnorm, h, tabular, label, W_fc, b_fc,
                 W_inst, b_inst, W_img, b_img, W_tab, b_tab, W_cls, b_cls):
    f32 = np.float32
    h = np.asarray(h, f32)
    W_fc = np.asarray(W_fc, f32)
    b_fc = np.asarray(b_fc, f32)

    Z = np.exp(s_full.astype(np.float64)).sum()
    M = (M_unnorm / Z).astype(f32)[None, :]                     # [1, D1]

    # global top/bottom-K candidate reduce (host side of the all-gather)
    order = np.argsort(-s_full, kind="stable")
    top_p = order[:K]
    top_n = np.argsort(s_full, kind="stable")[:K]
    ids = np.concatenate([top_p, top_n])
    h512_sel = np.maximum(h[ids] @ W_fc + b_fc, 0.0)            # [2K, D1] f32
    targets = np.concatenate([np.ones(K, np.int64), np.zeros(K, np.int64)])
    W_inst = np.asarray(W_inst, f32)
    b_inst = np.asarray(b_inst, f32)
    losses = []
    for cc in range(C):
        lg = (h512_sel @ W_inst[cc] + b_inst[cc]).astype(np.float64)
        lg -= lg.max(axis=1, keepdims=True)
        logp = lg - np.log(np.exp(lg).sum(axis=1, keepdims=True))
        losses.append(-np.mean(logp[np.arange(2 * K), targets]))
    lab = int(np.asarray(label))
    inst_loss = np.float32(losses[lab])

    tabular = np.asarray(tabular, f32)
    img_score = 1.0 / (1.0 + np.exp(-(M @ np.asarray(W_img, f32) + np.asarray(b_img, f32))))
    tl = (tabular @ np.asarray(W_tab, f32) + np.asarray(b_tab, f32)).astype(np.float64)
    tl -= tl.max()
    tab_score = (np.exp(tl) / np.exp(tl).sum()).astype(f32)
    concat = np.concatenate([(img_score * M).astype(f32), (tab_score * tabular).astype(f32)], axis=1)
    logits = (concat @ np.asarray(W_cls, f32) + np.asarray(b_cls, f32)).astype(f32)
    lx = logits.astype(np.float64) - logits.max()
    Y_prob = (np.exp(lx) / np.exp(lx).sum()).astype(f32)
    Y_hat = np.argmax(logits, axis=1).astype(np.int32)
    A_raw = s_full[None, :].astype(f32)
    return logits, Y_prob, Y_hat, A_raw, inst_loss


def kernel(h, tabular, label, W_fc, b_fc, Wa, ba, Wb, bb, Wc, bc,
           W_inst, b_inst, W_img, b_img, W_tab, b_tab, W_cls, b_cls):
    s_full, M_unnorm, _ = _run_device(h, W_fc, Wa, Wb, Wc, b_fc, ba, bb, bc)
    return _finish_host(s_full, M_unnorm, h, tabular, label, W_fc, b_fc,
                        W_inst, b_inst, W_img, b_img, W_tab, b_tab, W_cls, b_cls)


# revision 14
# speedup vs baseline: 1.1978x; 1.1978x over previous
"""CLAM-SB attention-MIL forward pass on 8 Trainium2 NeuronCores.

Strategy (sharding_hint: shard instance dim N across devices):
  - N=50000 rows of h are split 8 ways (6250/core, zero-padded to 6272=14*448).
  - The host pre-transposes each shard to hT [1024, 6272] so the contraction
    dim lands on SBUF partitions with no on-device transposes.
  - Each core streams 14 column-slices of 448 instances:
      h512T = relu(W_fc^T @ hT)            (4x 128-partition chunks, resident per slice)
      aT    = tanh(Wa^T @ h512T), bT = sigmoid(Wb^T @ h512T)
      s     = Wc^T @ (aT*bT)  -- computed with Wc replicated across 128 output
              partitions, so exp(s) is already broadcast for the pooling step
      Mpart += sum_n exp(s_n) * h512T[:, n]  (fused mul+reduce on DVE)
    Scores are tiny (~|s|<0.5) so unnormalized exp() is safe: no global max/
    normalization dependency on device; the host divides by Z = sum exp(s).
  - Outputs per core: raw scores (the A_raw shard) + the [512] pooled partial.
  - Host: merge scores -> global softmax norm, global top/bottom-64 ("all-gather
    k candidates and reduce"), recompute the 128 selected h512 rows in f32
    (tiny), instance loss + bag classifier heads.
"""
import os
import sys

import numpy as np

sys.path.insert(0, "/opt/trn_rl_repo")

N, L, D1, D2, K, C, T = 50000, 1024, 512, 256, 64, 2, 49
NCORES = 8
NV = N // NCORES            # 6250 valid instances per core
NS = 448                    # slice width (<=512 psum f32; >=256 for f32r full rate)
NSLICES = 14
NP = NS * NSLICES           # 6272 padded
KL = L // 128               # 8 k-chunks over the fc contraction
KD = D1 // 128              # 4 chunks over d1
KE = D2 // 128              # 2 chunks over d2
VALID_LAST = NV - (NSLICES - 1) * NS   # 426 valid cols in the final slice

# matmul dtype: "fp16" (default) or "bf16"
MM_MODE = os.environ.get("KERNEL_MM_MODE", "fp16")

_CACHE = {}


def _build_program():
    import concourse.bass as bass
    import concourse.bacc as bacc
    import concourse.mybir as mybir
    import concourse.tile as tile
    from concourse.mybir import dt

    f32 = dt.float32
    # fp16: full PE rate (1 cyc/row), legal on ACT/DVE (unlike float32r whose
    # producer-dtype rule forbids DVE/ACT in the chain), ~10-bit mantissa.
    DT = dt.bfloat16 if MM_MODE == "bf16" else dt.float16

    def mmap(ap):
        return ap

    AF = mybir.ActivationFunctionType
    Alu = mybir.AluOpType
    Axis = mybir.AxisListType

    nc = bacc.Bacc("TRN2", target_bir_lowering=False, debug=False)

    # Packed constants (one DMA per family; host does the chunk packing):
    #   Wfc_p [128, KL*D1]   col k*D1+j   = W_fc[k*128+p, j]
    #   Wa_p  [128, KD*D2]   col d*D2+j   = Wa[d*128+p, j]        (Wb same)
    #   WcR_p [128, KE*128]  col e*128+m  = 0.5*Wc[e*128+p]  (replicated, 0.5
    #        from the sigmoid-as-tanh identity folded into Wc)
    #   bias_p[128, 9] f32: cols 0-3 b_fc chunks, 4-5 ba chunks,
    #        6-7 0.5*bb chunks, 8 bc replicated down partitions
    #   eye   [128,128] f32 identity for the PE-transpose of the M result
    hT_d = nc.dram_tensor("hT", [L, NP], DT, kind="ExternalInput").ap()
    wfc_d = nc.dram_tensor("Wfc_p", [128, KL * D1], DT, kind="ExternalInput").ap()
    wa_d = nc.dram_tensor("Wa_p", [128, KD * D2], DT, kind="ExternalInput").ap()
    wb_d = nc.dram_tensor("Wb_p", [128, KD * D2], DT, kind="ExternalInput").ap()
    wcr_d = nc.dram_tensor("WcR_p", [128, KE * 128], DT, kind="ExternalInput").ap()
    bias_d = nc.dram_tensor("bias_p", [128, 9], f32, kind="ExternalInput").ap()
    eye_d = nc.dram_tensor("eye", [128, 128], f32, kind="ExternalInput").ap()
    s_out_d = nc.dram_tensor("s_out", [NSLICES, NS], f32, kind="ExternalOutput").ap()
    m_out_d = nc.dram_tensor("M_out", [KD, 128], f32, kind="ExternalOutput").ap()

    with tile.TileContext(nc) as tc:
        with (
            tc.tile_pool(name="consts", bufs=1) as consts,
            tc.tile_pool(name="io", bufs=2 * KL) as io,
            tc.tile_pool(name="h512p", bufs=2 * KD) as h512p,
            tc.tile_pool(name="abp", bufs=4) as abp,
            tc.tile_pool(name="wp", bufs=2) as wp,
            tc.tile_pool(name="accp", bufs=1) as accp,
            tc.tile_pool(name="psfc", bufs=3, space="PSUM") as psfc,
            tc.tile_pool(name="psab", bufs=4, space="PSUM") as psab,
            tc.tile_pool(name="pss", bufs=1, space="PSUM") as pss,
        ):
            # ---- constants (packed, few big DMAs; wfc first so slice 0 can start)
            wfc_all = consts.tile([128, KL * D1], DT, name="wfc_all")
            nc.sync.dma_start(out=wfc_all, in_=wfc_d[:, :])
            wa_all = consts.tile([128, KD * D2], DT, name="wa_all")
            nc.gpsimd.dma_start(out=wa_all, in_=wa_d[:, :])
            wb_all = consts.tile([128, KD * D2], DT, name="wb_all")
            nc.gpsimd.dma_start(out=wb_all, in_=wb_d[:, :])
            wcr_all = consts.tile([128, KE * 128], DT, name="wcr_all")
            nc.gpsimd.dma_start(out=wcr_all, in_=wcr_d[:, :])
            bias_all = consts.tile([128, 9], f32, name="bias_all")
            nc.gpsimd.dma_start(out=bias_all, in_=bias_d[:, :])
            eye_sb = consts.tile([128, 128], f32, name="eye_sb")
            nc.gpsimd.dma_start(out=eye_sb, in_=eye_d[:, :])

            wfc = [wfc_all[:, k * D1:(k + 1) * D1] for k in range(KL)]
            wa = [wa_all[:, d * D2:(d + 1) * D2] for d in range(KD)]
            wb = [wb_all[:, d * D2:(d + 1) * D2] for d in range(KD)]
            wc_rep = [wcr_all[:, e * 128:(e + 1) * 128] for e in range(KE)]
            bfc_c = [bias_all[:, d:d + 1] for d in range(KD)]
            ba_c = [bias_all[:, 4 + e:5 + e] for e in range(KE)]
            bbh_c = [bias_all[:, 6 + e:7 + e] for e in range(KE)]
            bc_rep = bias_all[:, 8:9]
            bc_sb = bias_all[0:1, 8:9]

            macc = []
            for d in range(KD):
                md = accp.tile([128, NSLICES], f32, name=f"macc{d}")
                macc.append(md)

            # ---- streaming over instance slices ----
            for j in range(NSLICES):
                ht = []
                for k in range(KL):
                    htk = io.tile([128, NS], DT, tag="ht")
                    # split DMA-trigger load across two engine queues
                    eng = nc.sync if k % 2 == 0 else nc.gpsimd
                    eng.dma_start(
                        out=htk, in_=hT_d[k * 128:(k + 1) * 128, j * NS:(j + 1) * NS]
                    )
                    ht.append(htk)

                h512 = []
                for d in range(KD):
                    ps = psfc.tile([128, NS], f32, tag="psfc")
                    for k in range(KL):
                        nc.tensor.matmul(
                            ps[:, :],
                            mmap(wfc[k][:, d * 128:(d + 1) * 128]),
                            mmap(ht[k][:, :]),
                            start=(k == 0),
                            stop=(k == KL - 1),
                        )
                    hd = h512p.tile([128, NS], DT, tag="h512")
                    nc.scalar.activation(hd[:, :], ps[:, :], AF.Relu, bias=bfc_c[d][:, :])
                    h512.append(hd)

                ab = []
                for e in range(KE):
                    psa = psab.tile([128, NS], f32, tag="psab")
                    for d in range(KD):
                        nc.tensor.matmul(
                            psa[:, :],
                            mmap(wa[d][:, e * 128:(e + 1) * 128]),
                            mmap(h512[d][:, :]),
                            start=(d == 0),
                            stop=(d == KD - 1),
                        )
                    at = abp.tile([128, NS], DT, tag="at")
                    nc.scalar.activation(at[:, :], psa[:, :], AF.Tanh, bias=ba_c[e][:, :])

                    psb = psab.tile([128, NS], f32, tag="psab")
                    for d in range(KD):
                        nc.tensor.matmul(
                            psb[:, :],
                            mmap(wb[d][:, e * 128:(e + 1) * 128]),
                            mmap(h512[d][:, :]),
                            start=(d == 0),
                            stop=(d == KD - 1),
                        )
                    # sigmoid(x) = 0.5*tanh(x/2) + 0.5: tanh lives in the same
                    # ACT table as relu/exp/identity, so no table reloads. The
                    # 0.5 scale is folded into WcR_p (host) and the +1 into the
                    # fused (bt+1)*at below.
                    bt = abp.tile([128, NS], DT, tag="bt")
                    nc.scalar.activation(
                        bt[:, :], psb[:, :], AF.Tanh, bias=bbh_c[e][:, :], scale=0.5
                    )

                    abe = abp.tile([128, NS], DT, tag="ab")
                    nc.vector.scalar_tensor_tensor(
                        out=abe[:, :], in0=bt[:, :], scalar=1.0, in1=at[:, :],
                        op0=Alu.add, op1=Alu.mult,
                    )
                    ab.append(abe)

                # s replicated across all 128 partitions
                ps_s = pss.tile([128, NS], f32, tag="pss")
                for e in range(KE):
                    nc.tensor.matmul(
                        ps_s[:, :],
                        mmap(wc_rep[e][:, :]),
                        mmap(ab[e][:, :]),
                        start=(e == 0),
                        stop=(e == KE - 1),
                    )
                srow = wp.tile([1, NS], f32, tag="srow")
                nc.scalar.activation(srow[:, :], ps_s[0:1, :], AF.Identity, bias=bc_sb[:, :])
                nc.sync.dma_start(out=s_out_d[j, :], in_=srow[:, :])

                wbc = wp.tile([128, NS], DT, tag="wbc")
                nc.scalar.activation(wbc[:, :], ps_s[:, :], AF.Exp, bias=bc_rep[:, :])
                if j == NSLICES - 1:
                    nc.vector.memset(wbc[:, VALID_LAST:], 0.0)

                for d in range(KD):
                    # fused multiply + free-dim-sum on DVE (TensorTensorReduce
                    # crashes this runtime; STT's accum_out does the same job)
                    prod = wp.tile([128, NS], f32, tag="prod")
                    nc.vector.scalar_tensor_tensor(
                        out=prod[:, :],
                        in0=h512[d][:, :],
                        scalar=1.0,
                        in1=wbc[:, :],
                        op0=Alu.mult,
                        op1=Alu.mult,
                        accum_out=macc[d][:, j:j + 1],
                    )

            # ---- final M reduction ----
            # reduce into [128, KD], PE-transpose to [KD, 128] so the store is
            # KD contiguous rows instead of a 128-descriptor partition scatter
            mfin = wp.tile([128, KD], f32, tag="mfin")
            for d in range(KD):
                nc.vector.tensor_reduce(mfin[:, d:d + 1], macc[d][:, :], axis=Axis.X, op=Alu.add)
            mps = pss.tile([KD, 128], f32, tag="pss")
            nc.tensor.transpose(mps[:, :], mfin[:, :], eye_sb[:, :])
            mrow = wp.tile([KD, 128], f32, tag="mrow")
            nc.scalar.activation(mrow[:, :], mps[:, :], AF.Identity, bias=0.0)
            nc.sync.dma_start(out=m_out_d[:, :], in_=mrow[:, :])

    nc.compile()
    return nc


def _get_program():
    if "nc" not in _CACHE:
        _CACHE["nc"] = _build_program()
    return _CACHE["nc"]


def _np_dt():
    if MM_MODE == "bf16":
        import ml_dtypes
        return ml_dtypes.bfloat16
    return np.float16


def _prepare_in_maps(h, W_fc, Wa, Wb, Wc, b_fc, ba, bb, bc):
    ndt = _np_dt()
    f32 = np.float32
    W_fc = np.asarray(W_fc, f32)
    Wa = np.asarray(Wa, f32)
    Wb = np.asarray(Wb, f32)
    Wc = np.asarray(Wc, f32).reshape(D2, 1)
    bias_p = np.zeros((128, 9), f32)
    bias_p[:, 0:KD] = np.asarray(b_fc, f32).reshape(KD, 128).T
    bias_p[:, 4:4 + KE] = np.asarray(ba, f32).reshape(KE, 128).T
    bias_p[:, 6:6 + KE] = 0.5 * np.asarray(bb, f32).reshape(KE, 128).T
    bias_p[:, 8] = np.asarray(bc, f32).reshape(-1)[0]
    wcr = np.concatenate(
        [np.tile(0.5 * Wc[e * 128:(e + 1) * 128], (1, 128)) for e in range(KE)], axis=1)
    base = {
        "Wfc_p": np.ascontiguousarray(
            W_fc.reshape(KL, 128, D1).transpose(1, 0, 2).reshape(128, KL * D1), dtype=ndt),
        "Wa_p": np.ascontiguousarray(
            Wa.reshape(KD, 128, D2).transpose(1, 0, 2).reshape(128, KD * D2), dtype=ndt),
        "Wb_p": np.ascontiguousarray(
            Wb.reshape(KD, 128, D2).transpose(1, 0, 2).reshape(128, KD * D2), dtype=ndt),
        "WcR_p": np.ascontiguousarray(wcr, dtype=ndt),
        "bias_p": bias_p,
        "eye": np.eye(128, dtype=f32),
    }
    in_maps = []
    hT = np.ascontiguousarray(np.asarray(h, dtype=np.float32).T)   # [L, N]
    for c in range(NCORES):
        hT_pad = np.zeros((L, NP), dtype=ndt)
        hT_pad[:, :NV] = hT[:, c * NV:(c + 1) * NV]
        m = dict(base)
        m["hT"] = hT_pad
        in_maps.append(m)
    return in_maps


def _run_device(h, W_fc, Wa, Wb, Wc, b_fc, ba, bb, bc, trace=False):
    from concourse.bass_utils import run_bass_kernel_spmd

    nc = _get_program()
    in_maps = _prepare_in_maps(h, W_fc, Wa, Wb, Wc, b_fc, ba, bb, bc)
    res = run_bass_kernel_spmd(nc, in_maps, list(range(NCORES)), trace=trace)
    s_full = np.zeros(N, np.float32)
    M_unnorm = np.zeros(D1, np.float64)
    for c in range(NCORES):
        out = res.results[c]
        s_full[c * NV:(c + 1) * NV] = out["s_out"].reshape(-1)[:NV]
        M_unnorm += out["M_out"].reshape(-1).astype(np.float64)
    return s_full, M_unnorm, res


def _finish_host(s_full, M_unnorm, h, tabular, label, W_fc, b_fc,
                 W_inst, b_inst, W_img, b_img, W_tab, b_tab, W_cls, b_cls):
    f32 = np.float32
    h = np.asarray(h, f32)
    W_fc = np.asarray(W_fc, f32)
    b_fc = np.asarray(b_fc, f32)

    Z = np.exp(s_full.astype(np.float64)).sum()
    M = (M_unnorm / Z).astype(f32)[None, :]                     # [1, D1]

    # global top/bottom-K candidate reduce (host side of the all-gather)
    order = np.argsort(-s_full, kind="stable")
    top_p = order[:K]
    top_n = np.argsort(s_full, kind="stable")[:K]
    ids = np.concatenate([top_p, top_n])
    h512_sel = np.maximum(h[ids] @ W_fc + b_fc, 0.0)            # [2K, D1] f32
    targets = np.concatenate([np.ones(K, np.int64), np.zeros(K, np.int64)])
    W_inst = np.asarray(W_inst, f32)
    b_inst = np.asarray(b_inst, f32)
    losses = []
    for cc in range(C):
        lg = (h512_sel @ W_inst[cc] + b_inst[cc]).astype(np.float64)
        lg -= lg.max(axis=1, keepdims=True)
        logp = lg - np.log(np.exp(lg).sum(axis=1, keepdims=True))
        losses.append(-np.mean(logp[np.arange(2 * K), targets]))
    lab = int(np.asarray(label))
    inst_loss = np.float32(losses[lab])

    tabular = np.asarray(tabular, f32)
    img_score = 1.0 / (1.0 + np.exp(-(M @ np.asarray(W_img, f32) + np.asarray(b_img, f32))))
    tl = (tabular @ np.asarray(W_tab, f32) + np.asarray(b_tab, f32)).astype(np.float64)
    tl -= tl.max()
    tab_score = (np.exp(tl) / np.exp(tl).sum()).astype(f32)
    concat = np.concatenate([(img_score * M).astype(f32), (tab_score * tabular).astype(f32)], axis=1)
    logits = (concat @ np.asarray(W_cls, f32) + np.asarray(b_cls, f32)).astype(f32)
    lx = logits.astype(np.float64) - logits.max()
    Y_prob = (np.exp(lx) / np.exp(lx).sum()).astype(f32)
    Y_hat = np.argmax(logits, axis=1).astype(np.int32)
    A_raw = s_full[None, :].astype(f32)
    return logits, Y_prob, Y_hat, A_raw, inst_loss


def kernel(h, tabular, label, W_fc, b_fc, Wa, ba, Wb, bb, Wc, bc,
           W_inst, b_inst, W_img, b_img, W_tab, b_tab, W_cls, b_cls):
    s_full, M_unnorm, _ = _run_device(h, W_fc, Wa, Wb, Wc, b_fc, ba, bb, bc)
    return _finish_host(s_full, M_unnorm, h, tabular, label, W_fc, b_fc,
                        W_inst, b_inst, W_img, b_img, W_tab, b_tab, W_cls, b_cls)


# revision 19
# speedup vs baseline: 1.2033x; 1.0045x over previous
"""CLAM-SB attention-MIL forward pass on 8 Trainium2 NeuronCores.

Strategy (sharding_hint: shard instance dim N across devices):
  - N=50000 rows of h are split 8 ways (6250/core, zero-padded to 6272=14*448).
  - The host pre-transposes each shard to hT [1024, 6272] so the contraction
    dim lands on SBUF partitions with no on-device transposes.
  - Each core streams 14 column-slices of 448 instances:
      h512T = relu(W_fc^T @ hT)            (4x 128-partition chunks, resident per slice)
      aT    = tanh(Wa^T @ h512T), bT = sigmoid(Wb^T @ h512T)
      s     = Wc^T @ (aT*bT)  -- computed with Wc replicated across 128 output
              partitions, so exp(s) is already broadcast for the pooling step
      Mpart += sum_n exp(s_n) * h512T[:, n]  (fused mul+reduce on DVE)
    Scores are tiny (~|s|<0.5) so unnormalized exp() is safe: no global max/
    normalization dependency on device; the host divides by Z = sum exp(s).
  - Outputs per core: raw scores (the A_raw shard) + the [512] pooled partial.
  - Host: merge scores -> global softmax norm, global top/bottom-64 ("all-gather
    k candidates and reduce"), recompute the 128 selected h512 rows in f32
    (tiny), instance loss + bag classifier heads.
"""
import os
import sys

import numpy as np

sys.path.insert(0, "/opt/trn_rl_repo")

N, L, D1, D2, K, C, T = 50000, 1024, 512, 256, 64, 2, 49
NCORES = 8
NV = N // NCORES            # 6250 valid instances per core
NS = 448                    # slice width (<=512 psum f32; >=256 for f32r full rate)
NSLICES = 14
NP = NS * NSLICES           # 6272 padded
KL = L // 128               # 8 k-chunks over the fc contraction
KD = D1 // 128              # 4 chunks over d1
KE = D2 // 128              # 2 chunks over d2
VALID_LAST = NV - (NSLICES - 1) * NS   # 426 valid cols in the final slice

# matmul dtype: "fp16" (default) or "bf16"
MM_MODE = os.environ.get("KERNEL_MM_MODE", "fp16")

_CACHE = {}


def _build_program():
    import concourse.bass as bass
    import concourse.bacc as bacc
    import concourse.mybir as mybir
    import concourse.tile as tile
    from concourse.mybir import dt

    f32 = dt.float32
    # fp16: full PE rate (1 cyc/row), legal on ACT/DVE (unlike float32r whose
    # producer-dtype rule forbids DVE/ACT in the chain), ~10-bit mantissa.
    DT = dt.bfloat16 if MM_MODE == "bf16" else dt.float16

    def mmap(ap):
        return ap

    AF = mybir.ActivationFunctionType
    Alu = mybir.AluOpType
    Axis = mybir.AxisListType

    nc = bacc.Bacc("TRN2", target_bir_lowering=False, debug=False)

    # Packed constants (one DMA per family; host does the chunk packing):
    #   Wfc_p [128, KL*D1]   col k*D1+j   = W_fc[k*128+p, j]
    #   Wa_p  [128, KD*D2]   col d*D2+j   = Wa[d*128+p, j]        (Wb same)
    #   WcR_p [128, KE*128]  col e*128+m  = 0.5*Wc[e*128+p]  (replicated, 0.5
    #        from the sigmoid-as-tanh identity folded into Wc)
    #   bias_p[128, 9] f32: cols 0-3 b_fc chunks, 4-5 ba chunks,
    #        6-7 0.5*bb chunks, 8 bc replicated down partitions
    #   eye   [128,128] f32 identity for the PE-transpose of the M result
    hT_d = nc.dram_tensor("hT", [L, NP], DT, kind="ExternalInput").ap()
    wfc_d = nc.dram_tensor("Wfc_p", [128, KL * D1], DT, kind="ExternalInput").ap()
    wa_d = nc.dram_tensor("Wa_p", [128, KD * D2], DT, kind="ExternalInput").ap()
    wb_d = nc.dram_tensor("Wb_p", [128, KD * D2], DT, kind="ExternalInput").ap()
    wcr_d = nc.dram_tensor("WcR_p", [128, KE * 128], DT, kind="ExternalInput").ap()
    bias_d = nc.dram_tensor("bias_p", [128, 9], f32, kind="ExternalInput").ap()
    eye_d = nc.dram_tensor("eye", [128, 128], f32, kind="ExternalInput").ap()
    s_out_d = nc.dram_tensor("s_out", [NSLICES, NS], f32, kind="ExternalOutput").ap()
    m_out_d = nc.dram_tensor("M_out", [KD, 128], f32, kind="ExternalOutput").ap()

    with tile.TileContext(nc) as tc:
        with (
            tc.tile_pool(name="consts", bufs=1) as consts,
            tc.tile_pool(name="io", bufs=2 * KL) as io,
            tc.tile_pool(name="h512p", bufs=2 * KD) as h512p,
            tc.tile_pool(name="abp", bufs=4) as abp,
            tc.tile_pool(name="wp", bufs=2) as wp,
            tc.tile_pool(name="accp", bufs=1) as accp,
            tc.tile_pool(name="psfc", bufs=3, space="PSUM") as psfc,
            tc.tile_pool(name="psab", bufs=4, space="PSUM") as psab,
            tc.tile_pool(name="pss", bufs=1, space="PSUM") as pss,
        ):
            # ---- constants (packed, few big DMAs; wfc first so slice 0 can start)
            # wfc rides the gpsimd queue so slice 0's ht loads own the sync queue
            wfc_all = consts.tile([128, KL * D1], DT, name="wfc_all")
            nc.gpsimd.dma_start(out=wfc_all, in_=wfc_d[:, :])
            wa_all = consts.tile([128, KD * D2], DT, name="wa_all")
            nc.gpsimd.dma_start(out=wa_all, in_=wa_d[:, :])
            wb_all = consts.tile([128, KD * D2], DT, name="wb_all")
            nc.gpsimd.dma_start(out=wb_all, in_=wb_d[:, :])
            wcr_all = consts.tile([128, KE * 128], DT, name="wcr_all")
            nc.gpsimd.dma_start(out=wcr_all, in_=wcr_d[:, :])
            bias_all = consts.tile([128, 9], f32, name="bias_all")
            nc.gpsimd.dma_start(out=bias_all, in_=bias_d[:, :])
            eye_sb = consts.tile([128, 128], f32, name="eye_sb")
            nc.gpsimd.dma_start(out=eye_sb, in_=eye_d[:, :])

            wfc = [wfc_all[:, k * D1:(k + 1) * D1] for k in range(KL)]
            wa = [wa_all[:, d * D2:(d + 1) * D2] for d in range(KD)]
            wb = [wb_all[:, d * D2:(d + 1) * D2] for d in range(KD)]
            wc_rep = [wcr_all[:, e * 128:(e + 1) * 128] for e in range(KE)]
            bfc_c = [bias_all[:, d:d + 1] for d in range(KD)]
            ba_c = [bias_all[:, 4 + e:5 + e] for e in range(KE)]
            bbh_c = [bias_all[:, 6 + e:7 + e] for e in range(KE)]
            bc_rep = bias_all[:, 8:9]
            bc_sb = bias_all[0:1, 8:9]

            macc = []
            for d in range(KD):
                md = accp.tile([128, NSLICES], f32, name=f"macc{d}")
                macc.append(md)

            # ---- streaming over instance slices ----
            for j in range(NSLICES):
                ht = []
                for k in range(KL):
                    htk = io.tile([128, NS], DT, tag="ht")
                    # split DMA-trigger load across two engine queues; slice 0
                    # goes all-sync (gpsimd is busy loading the constants)
                    eng = nc.sync if (j == 0 or k % 2 == 0) else nc.gpsimd
                    eng.dma_start(
                        out=htk, in_=hT_d[k * 128:(k + 1) * 128, j * NS:(j + 1) * NS]
                    )
                    ht.append(htk)

                h512 = []
                for d in range(KD):
                    ps = psfc.tile([128, NS], f32, tag="psfc")
                    for k in range(KL):
                        nc.tensor.matmul(
                            ps[:, :],
                            mmap(wfc[k][:, d * 128:(d + 1) * 128]),
                            mmap(ht[k][:, :]),
                            start=(k == 0),
                            stop=(k == KL - 1),
                        )
                    hd = h512p.tile([128, NS], DT, tag="h512")
                    if d < 2:
                        # relu split across ACT and DVE to balance engine load
                        nc.scalar.activation(hd[:, :], ps[:, :], AF.Relu, bias=bfc_c[d][:, :])
                    else:
                        nc.vector.tensor_scalar(
                            out=hd[:, :], in0=ps[:, :], scalar1=bfc_c[d][:, :],
                            scalar2=0.0, op0=Alu.add, op1=Alu.max,
                        )
                    h512.append(hd)

                ab = []
                for e in range(KE):
                    psa = psab.tile([128, NS], f32, tag="psab")
                    for d in range(KD):
                        nc.tensor.matmul(
                            psa[:, :],
                            mmap(wa[d][:, e * 128:(e + 1) * 128]),
                            mmap(h512[d][:, :]),
                            start=(d == 0),
                            stop=(d == KD - 1),
                        )
                    at = abp.tile([128, NS], DT, tag="at")
                    nc.scalar.activation(at[:, :], psa[:, :], AF.Tanh, bias=ba_c[e][:, :])

                    psb = psab.tile([128, NS], f32, tag="psab")
                    for d in range(KD):
                        nc.tensor.matmul(
                            psb[:, :],
                            mmap(wb[d][:, e * 128:(e + 1) * 128]),
                            mmap(h512[d][:, :]),
                            start=(d == 0),
                            stop=(d == KD - 1),
                        )
                    # sigmoid(x) = 0.5*tanh(x/2) + 0.5: tanh lives in the same
                    # ACT table as relu/exp/identity, so no table reloads. The
                    # 0.5 scale is folded into WcR_p (host) and the +1 into the
                    # fused (bt+1)*at below.
                    bt = abp.tile([128, NS], DT, tag="bt")
                    nc.scalar.activation(
                        bt[:, :], psb[:, :], AF.Tanh, bias=bbh_c[e][:, :], scale=0.5
                    )

                    abe = abp.tile([128, NS], DT, tag="ab")
                    nc.vector.scalar_tensor_tensor(
                        out=abe[:, :], in0=bt[:, :], scalar=1.0, in1=at[:, :],
                        op0=Alu.add, op1=Alu.mult,
                    )
                    ab.append(abe)

                # s replicated across all 128 partitions
                ps_s = pss.tile([128, NS], f32, tag="pss")
                for e in range(KE):
                    nc.tensor.matmul(
                        ps_s[:, :],
                        mmap(wc_rep[e][:, :]),
                        mmap(ab[e][:, :]),
                        start=(e == 0),
                        stop=(e == KE - 1),
                    )
                srow = wp.tile([1, NS], f32, tag="srow")
                nc.scalar.activation(srow[:, :], ps_s[0:1, :], AF.Identity, bias=bc_sb[:, :])
                nc.sync.dma_start(out=s_out_d[j, :], in_=srow[:, :])

                wbc = wp.tile([128, NS], DT, tag="wbc")
                nc.scalar.activation(wbc[:, :], ps_s[:, :], AF.Exp, bias=bc_rep[:, :])
                if j == NSLICES - 1:
                    nc.vector.memset(wbc[:, VALID_LAST:], 0.0)

                for d in range(KD):
                    # fused multiply + free-dim-sum (TensorTensorReduce crashes
                    # this runtime and Pool rejects the STT opcode, so all on DVE)
                    eng = nc.vector
                    prod = wp.tile([128, NS], f32, tag="prod")
                    eng.scalar_tensor_tensor(
                        out=prod[:, :],
                        in0=h512[d][:, :],
                        scalar=1.0,
                        in1=wbc[:, :],
                        op0=Alu.mult,
                        op1=Alu.mult,
                        accum_out=macc[d][:, j:j + 1],
                    )

            # ---- final M reduction ----
            # reduce into [128, KD], PE-transpose to [KD, 128] so the store is
            # KD contiguous rows instead of a 128-descriptor partition scatter
            mfin = wp.tile([128, KD], f32, tag="mfin")
            for d in range(KD):
                nc.vector.tensor_reduce(mfin[:, d:d + 1], macc[d][:, :], axis=Axis.X, op=Alu.add)
            mps = pss.tile([KD, 128], f32, tag="pss")
            nc.tensor.transpose(mps[:, :], mfin[:, :], eye_sb[:, :])
            mrow = wp.tile([KD, 128], f32, tag="mrow")
            nc.scalar.activation(mrow[:, :], mps[:, :], AF.Identity, bias=0.0)
            nc.sync.dma_start(out=m_out_d[:, :], in_=mrow[:, :])

    nc.compile()
    return nc


def _get_program():
    if "nc" not in _CACHE:
        _CACHE["nc"] = _build_program()
    return _CACHE["nc"]


def _np_dt():
    if MM_MODE == "bf16":
        import ml_dtypes
        return ml_dtypes.bfloat16
    return np.float16


def _prepare_in_maps(h, W_fc, Wa, Wb, Wc, b_fc, ba, bb, bc):
    ndt = _np_dt()
    f32 = np.float32
    W_fc = np.asarray(W_fc, f32)
    Wa = np.asarray(Wa, f32)
    Wb = np.asarray(Wb, f32)
    Wc = np.asarray(Wc, f32).reshape(D2, 1)
    bias_p = np.zeros((128, 9), f32)
    bias_p[:, 0:KD] = np.asarray(b_fc, f32).reshape(KD, 128).T
    bias_p[:, 4:4 + KE] = np.asarray(ba, f32).reshape(KE, 128).T
    bias_p[:, 6:6 + KE] = 0.5 * np.asarray(bb, f32).reshape(KE, 128).T
    bias_p[:, 8] = np.asarray(bc, f32).reshape(-1)[0]
    wcr = np.concatenate(
        [np.tile(0.5 * Wc[e * 128:(e + 1) * 128], (1, 128)) for e in range(KE)], axis=1)
    base = {
        "Wfc_p": np.ascontiguousarray(
            W_fc.reshape(KL, 128, D1).transpose(1, 0, 2).reshape(128, KL * D1), dtype=ndt),
        "Wa_p": np.ascontiguousarray(
            Wa.reshape(KD, 128, D2).transpose(1, 0, 2).reshape(128, KD * D2), dtype=ndt),
        "Wb_p": np.ascontiguousarray(
            Wb.reshape(KD, 128, D2).transpose(1, 0, 2).reshape(128, KD * D2), dtype=ndt),
        "WcR_p": np.ascontiguousarray(wcr, dtype=ndt),
        "bias_p": bias_p,
        "eye": np.eye(128, dtype=f32),
    }
    in_maps = []
    hT = np.ascontiguousarray(np.asarray(h, dtype=np.float32).T)   # [L, N]
    for c in range(NCORES):
        hT_pad = np.zeros((L, NP), dtype=ndt)
        hT_pad[:, :NV] = hT[:, c * NV:(c + 1) * NV]
        m = dict(base)
        m["hT"] = hT_pad
        in_maps.append(m)
    return in_maps


def _run_device(h, W_fc, Wa, Wb, Wc, b_fc, ba, bb, bc, trace=False):
    from concourse.bass_utils import run_bass_kernel_spmd

    nc = _get_program()
    in_maps = _prepare_in_maps(h, W_fc, Wa, Wb, Wc, b_fc, ba, bb, bc)
    res = run_bass_kernel_spmd(nc, in_maps, list(range(NCORES)), trace=trace)
    s_full = np.zeros(N, np.float32)
    M_unnorm = np.zeros(D1, np.float64)
    for c in range(NCORES):
        out = res.results[c]
        s_full[c * NV:(c + 1) * NV] = out["s_out"].reshape(-1)[:NV]
        M_unnorm += out["M_out"].reshape(-1).astype(np.float64)
    return s_full, M_unnorm, res


def _finish_host(s_full, M_unnorm, h, tabular, label, W_fc, b_fc,
                 W_inst, b_inst, W_img, b_img, W_tab, b_tab, W_cls, b_cls):
    f32 = np.float32
    h = np.asarray(h, f32)
    W_fc = np.asarray(W_fc, f32)
    b_fc = np.asarray(b_fc, f32)

    Z = np.exp(s_full.astype(np.float64)).sum()
    M = (M_unnorm / Z).astype(f32)[None, :]                     # [1, D1]

    # global top/bottom-K candidate reduce (host side of the all-gather)
    order = np.argsort(-s_full, kind="stable")
    top_p = order[:K]
    top_n = np.argsort(s_full, kind="stable")[:K]
    ids = np.concatenate([top_p, top_n])
    h512_sel = np.maximum(h[ids] @ W_fc + b_fc, 0.0)            # [2K, D1] f32
    targets = np.concatenate([np.ones(K, np.int64), np.zeros(K, np.int64)])
    W_inst = np.asarray(W_inst, f32)
    b_inst = np.asarray(b_inst, f32)
    losses = []
    for cc in range(C):
        lg = (h512_sel @ W_inst[cc] + b_inst[cc]).astype(np.float64)
        lg -= lg.max(axis=1, keepdims=True)
        logp = lg - np.log(np.exp(lg).sum(axis=1, keepdims=True))
        losses.append(-np.mean(logp[np.arange(2 * K), targets]))
    lab = int(np.asarray(label))
    inst_loss = np.float32(losses[lab])

    tabular = np.asarray(tabular, f32)
    img_score = 1.0 / (1.0 + np.exp(-(M @ np.asarray(W_img, f32) + np.asarray(b_img, f32))))
    tl = (tabular @ np.asarray(W_tab, f32) + np.asarray(b_tab, f32)).astype(np.float64)
    tl -= tl.max()
    tab_score = (np.exp(tl) / np.exp(tl).sum()).astype(f32)
    concat = np.concatenate([(img_score * M).astype(f32), (tab_score * tabular).astype(f32)], axis=1)
    logits = (concat @ np.asarray(W_cls, f32) + np.asarray(b_cls, f32)).astype(f32)
    lx = logits.astype(np.float64) - logits.max()
    Y_prob = (np.exp(lx) / np.exp(lx).sum()).astype(f32)
    Y_hat = np.argmax(logits, axis=1).astype(np.int32)
    A_raw = s_full[None, :].astype(f32)
    return logits, Y_prob, Y_hat, A_raw, inst_loss


def kernel(h, tabular, label, W_fc, b_fc, Wa, ba, Wb, bb, Wc, bc,
           W_inst, b_inst, W_img, b_img, W_tab, b_tab, W_cls, b_cls):
    s_full, M_unnorm, _ = _run_device(h, W_fc, Wa, Wb, Wc, b_fc, ba, bb, bc)
    return _finish_host(s_full, M_unnorm, h, tabular, label, W_fc, b_fc,
                        W_inst, b_inst, W_img, b_img, W_tab, b_tab, W_cls, b_cls)


# revision 23
# speedup vs baseline: 1.2193x; 1.0133x over previous
"""CLAM-SB attention-MIL forward pass on 8 Trainium2 NeuronCores.

Strategy (sharding_hint: shard instance dim N across devices):
  - N=50000 rows of h are split 8 ways (6250/core, zero-padded to 6272=14*448).
  - The host pre-transposes each shard to hT [1024, 6272] so the contraction
    dim lands on SBUF partitions with no on-device transposes.
  - Each core streams 14 column-slices of 448 instances:
      h512T = relu(W_fc^T @ hT)            (4x 128-partition chunks, resident per slice)
      aT    = tanh(Wa^T @ h512T), bT = sigmoid(Wb^T @ h512T)
      s     = Wc^T @ (aT*bT)  -- computed with Wc replicated across 128 output
              partitions, so exp(s) is already broadcast for the pooling step
      Mpart += sum_n exp(s_n) * h512T[:, n]  (fused mul+reduce on DVE)
    Scores are tiny (~|s|<0.5) so unnormalized exp() is safe: no global max/
    normalization dependency on device; the host divides by Z = sum exp(s).
  - Outputs per core: raw scores (the A_raw shard) + the [512] pooled partial.
  - Host: merge scores -> global softmax norm, global top/bottom-64 ("all-gather
    k candidates and reduce"), recompute the 128 selected h512 rows in f32
    (tiny), instance loss + bag classifier heads.
"""
import os
import sys

import numpy as np

sys.path.insert(0, "/opt/trn_rl_repo")

N, L, D1, D2, K, C, T = 50000, 1024, 512, 256, 64, 2, 49
NCORES = 8
NV = N // NCORES            # 6250 valid instances per core
NS = 448                    # slice width (<=512 psum f32; >=256 for f32r full rate)
NSLICES = 14
NP = NS * NSLICES           # 6272 padded
KL = L // 128               # 8 k-chunks over the fc contraction
KD = D1 // 128              # 4 chunks over d1
KE = D2 // 128              # 2 chunks over d2
VALID_LAST = NV - (NSLICES - 1) * NS   # 426 valid cols in the final slice

# matmul dtype: "fp16" (default) or "bf16"
MM_MODE = os.environ.get("KERNEL_MM_MODE", "fp16")

_CACHE = {}


def _build_program():
    import concourse.bass as bass
    import concourse.bacc as bacc
    import concourse.mybir as mybir
    import concourse.tile as tile
    from concourse.mybir import dt

    f32 = dt.float32
    # fp16: full PE rate (1 cyc/row), legal on ACT/DVE (unlike float32r whose
    # producer-dtype rule forbids DVE/ACT in the chain), ~10-bit mantissa.
    DT = dt.bfloat16 if MM_MODE == "bf16" else dt.float16

    def mmap(ap):
        return ap

    AF = mybir.ActivationFunctionType
    Alu = mybir.AluOpType
    Axis = mybir.AxisListType

    nc = bacc.Bacc("TRN2", target_bir_lowering=False, debug=False)

    # Packed constants (one DMA per family; host does the chunk packing):
    #   Wfc_p [128, KL*D1]   col k*D1+j   = W_fc[k*128+p, j]
    #   Wa_p  [128, KD*D2]   col d*D2+j   = Wa[d*128+p, j]        (Wb same)
    #   WcR_p [128, KE*128]  col e*128+m  = 0.5*Wc[e*128+p]  (replicated, 0.5
    #        from the sigmoid-as-tanh identity folded into Wc)
    #   bias_p[128, 9] f32: cols 0-3 b_fc chunks, 4-5 ba chunks,
    #        6-7 0.5*bb chunks, 8 bc replicated down partitions
    #   eye   [128,128] f32 identity for the PE-transpose of the M result
    hT_d = nc.dram_tensor("hT", [L, NP], DT, kind="ExternalInput").ap()
    wfc_d = nc.dram_tensor("Wfc_p", [128, KL * D1], DT, kind="ExternalInput").ap()
    wa_d = nc.dram_tensor("Wa_p", [128, KD * D2], DT, kind="ExternalInput").ap()
    wb_d = nc.dram_tensor("Wb_p", [128, KD * D2], DT, kind="ExternalInput").ap()
    wcr_d = nc.dram_tensor("WcR_p", [128, KE * 128], DT, kind="ExternalInput").ap()
    bias_d = nc.dram_tensor("bias_p", [128, 9], f32, kind="ExternalInput").ap()
    eye_d = nc.dram_tensor("eye", [128, 128], f32, kind="ExternalInput").ap()
    s_out_d = nc.dram_tensor("s_out", [NSLICES, NS], f32, kind="ExternalOutput").ap()
    m_out_d = nc.dram_tensor("M_out", [KD, 128], f32, kind="ExternalOutput").ap()

    with tile.TileContext(nc) as tc:
        with (
            tc.tile_pool(name="consts", bufs=1) as consts,
            tc.tile_pool(name="io", bufs=2 * KL) as io,
            tc.tile_pool(name="h512p", bufs=2 * KD) as h512p,
            tc.tile_pool(name="abp", bufs=4) as abp,
            tc.tile_pool(name="wp", bufs=2) as wp,
            tc.tile_pool(name="accp", bufs=1) as accp,
            tc.tile_pool(name="psfc", bufs=3, space="PSUM") as psfc,
            tc.tile_pool(name="psab", bufs=4, space="PSUM") as psab,
            tc.tile_pool(name="pss", bufs=1, space="PSUM") as pss,
        ):
            # ---- warm-up: preload the ACT function table and spin the PE so
            # HAM un-throttles to 2.4 GHz while the head DMAs are in flight
            scr0 = consts.tile([1, 1], f32, name="scr0")
            nc.vector.memset(scr0[:, :], 0.0)
            scr1 = consts.tile([1, 1], f32, name="scr1")
            nc.scalar.activation(scr1[:, :], scr0[:, :], AF.Relu)
            wsrc = consts.tile([128, NS], DT, name="wsrc")
            nc.vector.memset(wsrc[:, :], 0.0)
            for _ in range(12):
                wps = psfc.tile([128, NS], f32, tag="psfc")
                nc.tensor.matmul(wps[:, :], wsrc[:, 0:128], wsrc[:, :], start=True, stop=True)

            # ---- constants (packed, few big DMAs on otherwise-idle queues so
            # slice 0's ht loads own the sync+gpsimd queues)
            wfc_all = consts.tile([128, KL * D1], DT, name="wfc_all")
            nc.scalar.dma_start(out=wfc_all, in_=wfc_d[:, :])
            wa_all = consts.tile([128, KD * D2], DT, name="wa_all")
            nc.scalar.dma_start(out=wa_all, in_=wa_d[:, :])
            wb_all = consts.tile([128, KD * D2], DT, name="wb_all")
            nc.scalar.dma_start(out=wb_all, in_=wb_d[:, :])
            wcr_all = consts.tile([128, KE * 128], DT, name="wcr_all")
            nc.scalar.dma_start(out=wcr_all, in_=wcr_d[:, :])
            bias_all = consts.tile([128, 9], f32, name="bias_all")
            nc.scalar.dma_start(out=bias_all, in_=bias_d[:, :])
            eye_sb = consts.tile([128, 128], f32, name="eye_sb")
            nc.scalar.dma_start(out=eye_sb, in_=eye_d[:, :])

            wfc = [wfc_all[:, k * D1:(k + 1) * D1] for k in range(KL)]
            wa = [wa_all[:, d * D2:(d + 1) * D2] for d in range(KD)]
            wb = [wb_all[:, d * D2:(d + 1) * D2] for d in range(KD)]
            wc_rep = [wcr_all[:, e * 128:(e + 1) * 128] for e in range(KE)]
            bfc_c = [bias_all[:, d:d + 1] for d in range(KD)]
            ba_c = [bias_all[:, 4 + e:5 + e] for e in range(KE)]
            bbh_c = [bias_all[:, 6 + e:7 + e] for e in range(KE)]
            bc_rep = bias_all[:, 8:9]
            bc_sb = bias_all[0:1, 8:9]

            macc = []
            for d in range(KD):
                md = accp.tile([128, NSLICES], f32, name=f"macc{d}")
                macc.append(md)

            # ---- streaming over instance slices ----
            for j in range(NSLICES):
                ht = []
                for k in range(KL):
                    htk = io.tile([128, NS], DT, tag="ht")
                    # split DMA-trigger load across the sync+gpsimd queues
                    eng = nc.sync if k % 2 == 0 else nc.gpsimd
                    eng.dma_start(
                        out=htk, in_=hT_d[k * 128:(k + 1) * 128, j * NS:(j + 1) * NS]
                    )
                    ht.append(htk)

                h512 = []
                for d in range(KD):
                    ps = psfc.tile([128, NS], f32, tag="psfc")
                    for k in range(KL):
                        nc.tensor.matmul(
                            ps[:, :],
                            mmap(wfc[k][:, d * 128:(d + 1) * 128]),
                            mmap(ht[k][:, :]),
                            start=(k == 0),
                            stop=(k == KL - 1),
                        )
                    hd = h512p.tile([128, NS], DT, tag="h512")
                    if d < 2:
                        # relu split across ACT and DVE to balance engine load
                        nc.scalar.activation(hd[:, :], ps[:, :], AF.Relu, bias=bfc_c[d][:, :])
                    else:
                        nc.vector.tensor_scalar(
                            out=hd[:, :], in0=ps[:, :], scalar1=bfc_c[d][:, :],
                            scalar2=0.0, op0=Alu.add, op1=Alu.max,
                        )
                    h512.append(hd)

                ab = []
                for e in range(KE):
                    psa = psab.tile([128, NS], f32, tag="psab")
                    for d in range(KD):
                        nc.tensor.matmul(
                            psa[:, :],
                            mmap(wa[d][:, e * 128:(e + 1) * 128]),
                            mmap(h512[d][:, :]),
                            start=(d == 0),
                            stop=(d == KD - 1),
                        )
                    at = abp.tile([128, NS], DT, tag="at")
                    nc.scalar.activation(at[:, :], psa[:, :], AF.Tanh, bias=ba_c[e][:, :])

                    psb = psab.tile([128, NS], f32, tag="psab")
                    for d in range(KD):
                        nc.tensor.matmul(
                            psb[:, :],
                            mmap(wb[d][:, e * 128:(e + 1) * 128]),
                            mmap(h512[d][:, :]),
                            start=(d == 0),
                            stop=(d == KD - 1),
                        )
                    # sigmoid(x) = 0.5*tanh(x/2) + 0.5: tanh lives in the same
                    # ACT table as relu/exp/identity, so no table reloads. The
                    # 0.5 scale is folded into WcR_p (host) and the +1 into the
                    # fused (bt+1)*at below.
                    bt = abp.tile([128, NS], DT, tag="bt")
                    nc.scalar.activation(
                        bt[:, :], psb[:, :], AF.Tanh, bias=bbh_c[e][:, :], scale=0.5
                    )

                    abe = abp.tile([128, NS], DT, tag="ab")
                    nc.vector.scalar_tensor_tensor(
                        out=abe[:, :], in0=bt[:, :], scalar=1.0, in1=at[:, :],
                        op0=Alu.add, op1=Alu.mult,
                    )
                    ab.append(abe)

                # s replicated across all 128 partitions
                ps_s = pss.tile([128, NS], f32, tag="pss")
                for e in range(KE):
                    nc.tensor.matmul(
                        ps_s[:, :],
                        mmap(wc_rep[e][:, :]),
                        mmap(ab[e][:, :]),
                        start=(e == 0),
                        stop=(e == KE - 1),
                    )
                srow = wp.tile([1, NS], f32, tag="srow")
                nc.scalar.activation(srow[:, :], ps_s[0:1, :], AF.Identity, bias=bc_sb[:, :])
                nc.sync.dma_start(out=s_out_d[j, :], in_=srow[:, :])

                wbc = wp.tile([128, NS], DT, tag="wbc")
                nc.scalar.activation(wbc[:, :], ps_s[:, :], AF.Exp, bias=bc_rep[:, :])
                if j == NSLICES - 1:
                    nc.vector.memset(wbc[:, VALID_LAST:], 0.0)

                for d in range(KD):
                    # fused multiply + free-dim-sum (TensorTensorReduce crashes
                    # this runtime and Pool rejects the STT opcode, so all on DVE)
                    eng = nc.vector
                    prod = wp.tile([128, NS], f32, tag="prod")
                    eng.scalar_tensor_tensor(
                        out=prod[:, :],
                        in0=h512[d][:, :],
                        scalar=1.0,
                        in1=wbc[:, :],
                        op0=Alu.mult,
                        op1=Alu.mult,
                        accum_out=macc[d][:, j:j + 1],
                    )

            # ---- final M reduction ----
            # reduce into [128, KD], PE-transpose to [KD, 128] so the store is
            # KD contiguous rows instead of a 128-descriptor partition scatter
            mfin = wp.tile([128, KD], f32, tag="mfin")
            for d in range(KD):
                nc.vector.tensor_reduce(mfin[:, d:d + 1], macc[d][:, :], axis=Axis.X, op=Alu.add)
            mps = pss.tile([KD, 128], f32, tag="pss")
            nc.tensor.transpose(mps[:, :], mfin[:, :], eye_sb[:, :])
            mrow = wp.tile([KD, 128], f32, tag="mrow")
            nc.vector.tensor_copy(mrow[:, :], mps[:, :])
            nc.sync.dma_start(out=m_out_d[:, :], in_=mrow[:, :])

    nc.compile()
    return nc


def _get_program():
    if "nc" not in _CACHE:
        _CACHE["nc"] = _build_program()
    return _CACHE["nc"]


def _np_dt():
    if MM_MODE == "bf16":
        import ml_dtypes
        return ml_dtypes.bfloat16
    return np.float16


def _prepare_in_maps(h, W_fc, Wa, Wb, Wc, b_fc, ba, bb, bc):
    ndt = _np_dt()
    f32 = np.float32
    W_fc = np.asarray(W_fc, f32)
    Wa = np.asarray(Wa, f32)
    Wb = np.asarray(Wb, f32)
    Wc = np.asarray(Wc, f32).reshape(D2, 1)
    bias_p = np.zeros((128, 9), f32)
    bias_p[:, 0:KD] = np.asarray(b_fc, f32).reshape(KD, 128).T
    bias_p[:, 4:4 + KE] = np.asarray(ba, f32).reshape(KE, 128).T
    bias_p[:, 6:6 + KE] = 0.5 * np.asarray(bb, f32).reshape(KE, 128).T
    bias_p[:, 8] = np.asarray(bc, f32).reshape(-1)[0]
    wcr = np.concatenate(
        [np.tile(0.5 * Wc[e * 128:(e + 1) * 128], (1, 128)) for e in range(KE)], axis=1)
    base = {
        "Wfc_p": np.ascontiguousarray(
            W_fc.reshape(KL, 128, D1).transpose(1, 0, 2).reshape(128, KL * D1), dtype=ndt),
        "Wa_p": np.ascontiguousarray(
            Wa.reshape(KD, 128, D2).transpose(1, 0, 2).reshape(128, KD * D2), dtype=ndt),
        "Wb_p": np.ascontiguousarray(
            Wb.reshape(KD, 128, D2).transpose(1, 0, 2).reshape(128, KD * D2), dtype=ndt),
        "WcR_p": np.ascontiguousarray(wcr, dtype=ndt),
        "bias_p": bias_p,
        "eye": np.eye(128, dtype=f32),
    }
    in_maps = []
    hT = np.ascontiguousarray(np.asarray(h, dtype=np.float32).T)   # [L, N]
    for c in range(NCORES):
        hT_pad = np.zeros((L, NP), dtype=ndt)
        hT_pad[:, :NV] = hT[:, c * NV:(c + 1) * NV]
        m = dict(base)
        m["hT"] = hT_pad
        in_maps.append(m)
    return in_maps


def _run_device(h, W_fc, Wa, Wb, Wc, b_fc, ba, bb, bc, trace=False):
    from concourse.bass_utils import run_bass_kernel_spmd

    nc = _get_program()
    in_maps = _prepare_in_maps(h, W_fc, Wa, Wb, Wc, b_fc, ba, bb, bc)
    res = run_bass_kernel_spmd(nc, in_maps, list(range(NCORES)), trace=trace)
    s_full = np.zeros(N, np.float32)
    M_unnorm = np.zeros(D1, np.float64)
    for c in range(NCORES):
        out = res.results[c]
        s_full[c * NV:(c + 1) * NV] = out["s_out"].reshape(-1)[:NV]
        M_unnorm += out["M_out"].reshape(-1).astype(np.float64)
    return s_full, M_unnorm, res


def _finish_host(s_full, M_unnorm, h, tabular, label, W_fc, b_fc,
                 W_inst, b_inst, W_img, b_img, W_tab, b_tab, W_cls, b_cls):
    f32 = np.float32
    h = np.asarray(h, f32)
    W_fc = np.asarray(W_fc, f32)
    b_fc = np.asarray(b_fc, f32)

    Z = np.exp(s_full.astype(np.float64)).sum()
    M = (M_unnorm / Z).astype(f32)[None, :]                     # [1, D1]

    # global top/bottom-K candidate reduce (host side of the all-gather)
    order = np.argsort(-s_full, kind="stable")
    top_p = order[:K]
    top_n = np.argsort(s_full, kind="stable")[:K]
    ids = np.concatenate([top_p, top_n])
    h512_sel = np.maximum(h[ids] @ W_fc + b_fc, 0.0)            # [2K, D1] f32
    targets = np.concatenate([np.ones(K, np.int64), np.zeros(K, np.int64)])
    W_inst = np.asarray(W_inst, f32)
    b_inst = np.asarray(b_inst, f32)
    losses = []
    for cc in range(C):
        lg = (h512_sel @ W_inst[cc] + b_inst[cc]).astype(np.float64)
        lg -= lg.max(axis=1, keepdims=True)
        logp = lg - np.log(np.exp(lg).sum(axis=1, keepdims=True))
        losses.append(-np.mean(logp[np.arange(2 * K), targets]))
    lab = int(np.asarray(label))
    inst_loss = np.float32(losses[lab])

    tabular = np.asarray(tabular, f32)
    img_score = 1.0 / (1.0 + np.exp(-(M @ np.asarray(W_img, f32) + np.asarray(b_img, f32))))
    tl = (tabular @ np.asarray(W_tab, f32) + np.asarray(b_tab, f32)).astype(np.float64)
    tl -= tl.max()
    tab_score = (np.exp(tl) / np.exp(tl).sum()).astype(f32)
    concat = np.concatenate([(img_score * M).astype(f32), (tab_score * tabular).astype(f32)], axis=1)
    logits = (concat @ np.asarray(W_cls, f32) + np.asarray(b_cls, f32)).astype(f32)
    lx = logits.astype(np.float64) - logits.max()
    Y_prob = (np.exp(lx) / np.exp(lx).sum()).astype(f32)
    Y_hat = np.argmax(logits, axis=1).astype(np.int32)
    A_raw = s_full[None, :].astype(f32)
    return logits, Y_prob, Y_hat, A_raw, inst_loss


def kernel(h, tabular, label, W_fc, b_fc, Wa, ba, Wb, bb, Wc, bc,
           W_inst, b_inst, W_img, b_img, W_tab, b_tab, W_cls, b_cls):
    s_full, M_unnorm, _ = _run_device(h, W_fc, Wa, Wb, Wc, b_fc, ba, bb, bc)
    return _finish_host(s_full, M_unnorm, h, tabular, label, W_fc, b_fc,
                        W_inst, b_inst, W_img, b_img, W_tab, b_tab, W_cls, b_cls)


# revision 25
# speedup vs baseline: 1.2263x; 1.0058x over previous
"""CLAM-SB attention-MIL forward pass on 8 Trainium2 NeuronCores.

Strategy (sharding_hint: shard instance dim N across devices):
  - N=50000 rows of h are split 8 ways (6250/core, zero-padded to 6272=14*448).
  - The host pre-transposes each shard to hT [1024, 6272] so the contraction
    dim lands on SBUF partitions with no on-device transposes.
  - Each core streams 14 column-slices of 448 instances:
      h512T = relu(W_fc^T @ hT)            (4x 128-partition chunks, resident per slice)
      aT    = tanh(Wa^T @ h512T), bT = sigmoid(Wb^T @ h512T)
      s     = Wc^T @ (aT*bT)  -- computed with Wc replicated across 128 output
              partitions, so exp(s) is already broadcast for the pooling step
      Mpart += sum_n exp(s_n) * h512T[:, n]  (fused mul+reduce on DVE)
    Scores are tiny (~|s|<0.5) so unnormalized exp() is safe: no global max/
    normalization dependency on device; the host divides by Z = sum exp(s).
  - Outputs per core: raw scores (the A_raw shard) + the [512] pooled partial.
  - Host: merge scores -> global softmax norm, global top/bottom-64 ("all-gather
    k candidates and reduce"), recompute the 128 selected h512 rows in f32
    (tiny), instance loss + bag classifier heads.
"""
import os
import sys

import numpy as np

sys.path.insert(0, "/opt/trn_rl_repo")

N, L, D1, D2, K, C, T = 50000, 1024, 512, 256, 64, 2, 49
NCORES = 8
NV = N // NCORES            # 6250 valid instances per core
NS = 448                    # slice width (<=512 psum f32; >=256 for f32r full rate)
NSLICES = 14
NP = NS * NSLICES           # 6272 padded
KL = L // 128               # 8 k-chunks over the fc contraction
KD = D1 // 128              # 4 chunks over d1
KE = D2 // 128              # 2 chunks over d2
VALID_LAST = NV - (NSLICES - 1) * NS   # 426 valid cols in the final slice

# matmul dtype: "fp16" (default) or "bf16"
MM_MODE = os.environ.get("KERNEL_MM_MODE", "fp16")

_CACHE = {}


def _build_program():
    import concourse.bass as bass
    import concourse.bacc as bacc
    import concourse.mybir as mybir
    import concourse.tile as tile
    from concourse.mybir import dt

    f32 = dt.float32
    # fp16: full PE rate (1 cyc/row), legal on ACT/DVE (unlike float32r whose
    # producer-dtype rule forbids DVE/ACT in the chain), ~10-bit mantissa.
    DT = dt.bfloat16 if MM_MODE == "bf16" else dt.float16

    def mmap(ap):
        return ap

    AF = mybir.ActivationFunctionType
    Alu = mybir.AluOpType
    Axis = mybir.AxisListType

    nc = bacc.Bacc("TRN2", target_bir_lowering=False, debug=False)

    # Packed constants (one DMA per family; host does the chunk packing):
    #   Wfc_p [128, KL*D1]   col k*D1+j   = W_fc[k*128+p, j]
    #   Wa_p  [128, KD*D2]   col d*D2+j   = Wa[d*128+p, j]        (Wb same)
    #   WcR_p [128, KE*128]  col e*128+m  = 0.5*Wc[e*128+p]  (replicated, 0.5
    #        from the sigmoid-as-tanh identity folded into Wc)
    #   bias_p[128, 9] f32: cols 0-3 b_fc chunks, 4-5 ba chunks,
    #        6-7 0.5*bb chunks, 8 bc replicated down partitions
    #   eye   [128,128] f32 identity for the PE-transpose of the M result
    hT_d = nc.dram_tensor("hT", [L, NP], DT, kind="ExternalInput").ap()
    wfc_d = nc.dram_tensor("Wfc_p", [128, KL * D1], DT, kind="ExternalInput").ap()
    wa_d = nc.dram_tensor("Wa_p", [128, KD * D2], DT, kind="ExternalInput").ap()
    wb_d = nc.dram_tensor("Wb_p", [128, KD * D2], DT, kind="ExternalInput").ap()
    wcr_d = nc.dram_tensor("WcR_p", [128, KE * 128], DT, kind="ExternalInput").ap()
    bias_d = nc.dram_tensor("bias_p", [128, 9], f32, kind="ExternalInput").ap()
    eye_d = nc.dram_tensor("eye", [128, 128], f32, kind="ExternalInput").ap()
    s_out_d = nc.dram_tensor("s_out", [NSLICES, NS], f32, kind="ExternalOutput").ap()
    m_out_d = nc.dram_tensor("M_out", [KD, 128], f32, kind="ExternalOutput").ap()

    with tile.TileContext(nc) as tc:
        with (
            tc.tile_pool(name="consts", bufs=1) as consts,
            tc.tile_pool(name="io", bufs=2 * KL) as io,
            tc.tile_pool(name="h512p", bufs=2 * KD) as h512p,
            tc.tile_pool(name="abp", bufs=4) as abp,
            tc.tile_pool(name="wp", bufs=2) as wp,
            tc.tile_pool(name="accp", bufs=1) as accp,
            tc.tile_pool(name="psfc", bufs=3, space="PSUM") as psfc,
            tc.tile_pool(name="psab", bufs=4, space="PSUM") as psab,
            tc.tile_pool(name="pss", bufs=1, space="PSUM") as pss,
        ):
            # ---- warm-up: preload the ACT function table and spin the PE so
            # HAM un-throttles to 2.4 GHz while the head DMAs are in flight
            scr0 = consts.tile([1, 1], f32, name="scr0")
            nc.vector.memset(scr0[:, :], 0.0)
            scr1 = consts.tile([1, 1], f32, name="scr1")
            nc.scalar.activation(scr1[:, :], scr0[:, :], AF.Relu)
            wsrc = consts.tile([128, NS], DT, name="wsrc")
            nc.vector.memset(wsrc[:, :], 0.0)
            for _ in range(12):
                wps = psfc.tile([128, NS], f32, tag="psfc")
                nc.tensor.matmul(wps[:, :], wsrc[:, 0:128], wsrc[:, :], start=True, stop=True)

            # ---- constants (packed, few big DMAs on otherwise-idle queues so
            # slice 0's ht loads own the sync+gpsimd queues)
            wfc_all = consts.tile([128, KL * D1], DT, name="wfc_all")
            half = KL * D1 // 2
            nc.scalar.dma_start(out=wfc_all[:, :half], in_=wfc_d[:, :half])
            nc.scalar.dma_start(out=wfc_all[:, half:], in_=wfc_d[:, half:])
            wa_all = consts.tile([128, KD * D2], DT, name="wa_all")
            nc.scalar.dma_start(out=wa_all, in_=wa_d[:, :])
            wb_all = consts.tile([128, KD * D2], DT, name="wb_all")
            nc.scalar.dma_start(out=wb_all, in_=wb_d[:, :])
            wcr_all = consts.tile([128, KE * 128], DT, name="wcr_all")
            nc.scalar.dma_start(out=wcr_all, in_=wcr_d[:, :])
            bias_all = consts.tile([128, 9], f32, name="bias_all")
            nc.scalar.dma_start(out=bias_all, in_=bias_d[:, :])
            eye_sb = consts.tile([128, 128], f32, name="eye_sb")
            nc.scalar.dma_start(out=eye_sb, in_=eye_d[:, :])

            wfc = [wfc_all[:, k * D1:(k + 1) * D1] for k in range(KL)]
            wa = [wa_all[:, d * D2:(d + 1) * D2] for d in range(KD)]
            wb = [wb_all[:, d * D2:(d + 1) * D2] for d in range(KD)]
            wc_rep = [wcr_all[:, e * 128:(e + 1) * 128] for e in range(KE)]
            bfc_c = [bias_all[:, d:d + 1] for d in range(KD)]
            ba_c = [bias_all[:, 4 + e:5 + e] for e in range(KE)]
            bbh_c = [bias_all[:, 6 + e:7 + e] for e in range(KE)]
            bc_rep = bias_all[:, 8:9]
            bc_sb = bias_all[0:1, 8:9]

            macc = []
            for d in range(KD):
                md = accp.tile([128, NSLICES], f32, name=f"macc{d}")
                macc.append(md)

            # ---- streaming over instance slices ----
            for j in range(NSLICES):
                ht = []
                for k in range(KL):
                    htk = io.tile([128, NS], DT, tag="ht")
                    # split DMA-trigger load across the sync+gpsimd queues
                    eng = nc.sync if k % 2 == 0 else nc.gpsimd
                    eng.dma_start(
                        out=htk, in_=hT_d[k * 128:(k + 1) * 128, j * NS:(j + 1) * NS]
                    )
                    ht.append(htk)

                h512 = []
                for d in range(KD):
                    ps = psfc.tile([128, NS], f32, tag="psfc")
                    for k in range(KL):
                        nc.tensor.matmul(
                            ps[:, :],
                            mmap(wfc[k][:, d * 128:(d + 1) * 128]),
                            mmap(ht[k][:, :]),
                            start=(k == 0),
                            stop=(k == KL - 1),
                        )
                    hd = h512p.tile([128, NS], DT, tag="h512")
                    if d < 2:
                        # relu split across ACT and DVE to balance engine load
                        nc.scalar.activation(hd[:, :], ps[:, :], AF.Relu, bias=bfc_c[d][:, :])
                    else:
                        nc.vector.tensor_scalar(
                            out=hd[:, :], in0=ps[:, :], scalar1=bfc_c[d][:, :],
                            scalar2=0.0, op0=Alu.add, op1=Alu.max,
                        )
                    h512.append(hd)

                ab = []
                for e in range(KE):
                    psa = psab.tile([128, NS], f32, tag="psab")
                    for d in range(KD):
                        nc.tensor.matmul(
                            psa[:, :],
                            mmap(wa[d][:, e * 128:(e + 1) * 128]),
                            mmap(h512[d][:, :]),
                            start=(d == 0),
                            stop=(d == KD - 1),
                        )
                    at = abp.tile([128, NS], DT, tag="at")
                    nc.scalar.activation(at[:, :], psa[:, :], AF.Tanh, bias=ba_c[e][:, :])

                    psb = psab.tile([128, NS], f32, tag="psab")
                    for d in range(KD):
                        nc.tensor.matmul(
                            psb[:, :],
                            mmap(wb[d][:, e * 128:(e + 1) * 128]),
                            mmap(h512[d][:, :]),
                            start=(d == 0),
                            stop=(d == KD - 1),
                        )
                    # sigmoid(x) = 0.5*tanh(x/2) + 0.5: tanh lives in the same
                    # ACT table as relu/exp/identity, so no table reloads. The
                    # 0.5 scale is folded into WcR_p (host) and the +1 into the
                    # fused (bt+1)*at below.
                    bt = abp.tile([128, NS], DT, tag="bt")
                    nc.scalar.activation(
                        bt[:, :], psb[:, :], AF.Tanh, bias=bbh_c[e][:, :], scale=0.5
                    )

                    abe = abp.tile([128, NS], DT, tag="ab")
                    nc.vector.scalar_tensor_tensor(
                        out=abe[:, :], in0=bt[:, :], scalar=1.0, in1=at[:, :],
                        op0=Alu.add, op1=Alu.mult,
                    )
                    ab.append(abe)

                # s replicated across all 128 partitions
                ps_s = pss.tile([128, NS], f32, tag="pss")
                for e in range(KE):
                    nc.tensor.matmul(
                        ps_s[:, :],
                        mmap(wc_rep[e][:, :]),
                        mmap(ab[e][:, :]),
                        start=(e == 0),
                        stop=(e == KE - 1),
                    )
                srow = wp.tile([1, NS], f32, tag="srow")
                nc.scalar.activation(srow[:, :], ps_s[0:1, :], AF.Identity, bias=bc_sb[:, :])
                nc.sync.dma_start(out=s_out_d[j, :], in_=srow[:, :])

                wbc = wp.tile([128, NS], DT, tag="wbc")
                nc.scalar.activation(wbc[:, :], ps_s[:, :], AF.Exp, bias=bc_rep[:, :])
                if j == NSLICES - 1:
                    nc.vector.memset(wbc[:, VALID_LAST:], 0.0)

                for d in range(KD):
                    # fused multiply + free-dim-sum (TensorTensorReduce crashes
                    # this runtime and Pool rejects the STT opcode, so all on DVE)
                    eng = nc.vector
                    prod = wp.tile([128, NS], f32, tag="prod")
                    eng.scalar_tensor_tensor(
                        out=prod[:, :],
                        in0=h512[d][:, :],
                        scalar=1.0,
                        in1=wbc[:, :],
                        op0=Alu.mult,
                        op1=Alu.mult,
                        accum_out=macc[d][:, j:j + 1],
                    )

            # ---- final M reduction ----
            # reduce into [128, KD], PE-transpose to [KD, 128] so the store is
            # KD contiguous rows instead of a 128-descriptor partition scatter
            mfin = wp.tile([128, KD], f32, tag="mfin")
            for d in range(KD):
                nc.vector.tensor_reduce(mfin[:, d:d + 1], macc[d][:, :], axis=Axis.X, op=Alu.add)
            mps = pss.tile([KD, 128], f32, tag="pss")
            nc.tensor.transpose(mps[:, :], mfin[:, :], eye_sb[:, :])
            mrow = wp.tile([KD, 128], f32, tag="mrow")
            nc.vector.tensor_copy(mrow[:, :], mps[:, :])
            nc.sync.dma_start(out=m_out_d[:, :], in_=mrow[:, :])

    nc.compile()
    return nc


def _get_program():
    if "nc" not in _CACHE:
        _CACHE["nc"] = _build_program()
    return _CACHE["nc"]


def _np_dt():
    if MM_MODE == "bf16":
        import ml_dtypes
        return ml_dtypes.bfloat16
    return np.float16


def _prepare_in_maps(h, W_fc, Wa, Wb, Wc, b_fc, ba, bb, bc):
    ndt = _np_dt()
    f32 = np.float32
    W_fc = np.asarray(W_fc, f32)
    Wa = np.asarray(Wa, f32)
    Wb = np.asarray(Wb, f32)
    Wc = np.asarray(Wc, f32).reshape(D2, 1)
    bias_p = np.zeros((128, 9), f32)
    bias_p[:, 0:KD] = np.asarray(b_fc, f32).reshape(KD, 128).T
    bias_p[:, 4:4 + KE] = np.asarray(ba, f32).reshape(KE, 128).T
    bias_p[:, 6:6 + KE] = 0.5 * np.asarray(bb, f32).reshape(KE, 128).T
    bias_p[:, 8] = np.asarray(bc, f32).reshape(-1)[0]
    wcr = np.concatenate(
        [np.tile(0.5 * Wc[e * 128:(e + 1) * 128], (1, 128)) for e in range(KE)], axis=1)
    base = {
        "Wfc_p": np.ascontiguousarray(
            W_fc.reshape(KL, 128, D1).transpose(1, 0, 2).reshape(128, KL * D1), dtype=ndt),
        "Wa_p": np.ascontiguousarray(
            Wa.reshape(KD, 128, D2).transpose(1, 0, 2).reshape(128, KD * D2), dtype=ndt),
        "Wb_p": np.ascontiguousarray(
            Wb.reshape(KD, 128, D2).transpose(1, 0, 2).reshape(128, KD * D2), dtype=ndt),
        "WcR_p": np.ascontiguousarray(wcr, dtype=ndt),
        "bias_p": bias_p,
        "eye": np.eye(128, dtype=f32),
    }
    in_maps = []
    hT = np.ascontiguousarray(np.asarray(h, dtype=np.float32).T)   # [L, N]
    for c in range(NCORES):
        hT_pad = np.zeros((L, NP), dtype=ndt)
        hT_pad[:, :NV] = hT[:, c * NV:(c + 1) * NV]
        m = dict(base)
        m["hT"] = hT_pad
        in_maps.append(m)
    return in_maps


def _run_device(h, W_fc, Wa, Wb, Wc, b_fc, ba, bb, bc, trace=False, in_maps=None):
    from concourse.bass_utils import run_bass_kernel_spmd

    nc = _get_program()
    if in_maps is None:
        in_maps = _prepare_in_maps(h, W_fc, Wa, Wb, Wc, b_fc, ba, bb, bc)
    res = run_bass_kernel_spmd(nc, in_maps, list(range(NCORES)), trace=trace)
    s_full = np.zeros(N, np.float32)
    M_unnorm = np.zeros(D1, np.float64)
    for c in range(NCORES):
        out = res.results[c]
        s_full[c * NV:(c + 1) * NV] = out["s_out"].reshape(-1)[:NV]
        M_unnorm += out["M_out"].reshape(-1).astype(np.float64)
    return s_full, M_unnorm, res


def _finish_host(s_full, M_unnorm, h, tabular, label, W_fc, b_fc,
                 W_inst, b_inst, W_img, b_img, W_tab, b_tab, W_cls, b_cls):
    f32 = np.float32
    h = np.asarray(h, f32)
    W_fc = np.asarray(W_fc, f32)
    b_fc = np.asarray(b_fc, f32)

    Z = np.exp(s_full.astype(np.float64)).sum()
    M = (M_unnorm / Z).astype(f32)[None, :]                     # [1, D1]

    # global top/bottom-K candidate reduce (host side of the all-gather)
    order = np.argsort(-s_full, kind="stable")
    top_p = order[:K]
    top_n = np.argsort(s_full, kind="stable")[:K]
    ids = np.concatenate([top_p, top_n])
    h512_sel = np.maximum(h[ids] @ W_fc + b_fc, 0.0)            # [2K, D1] f32
    targets = np.concatenate([np.ones(K, np.int64), np.zeros(K, np.int64)])
    W_inst = np.asarray(W_inst, f32)
    b_inst = np.asarray(b_inst, f32)
    losses = []
    for cc in range(C):
        lg = (h512_sel @ W_inst[cc] + b_inst[cc]).astype(np.float64)
        lg -= lg.max(axis=1, keepdims=True)
        logp = lg - np.log(np.exp(lg).sum(axis=1, keepdims=True))
        losses.append(-np.mean(logp[np.arange(2 * K), targets]))
    lab = int(np.asarray(label))
    inst_loss = np.float32(losses[lab])

    tabular = np.asarray(tabular, f32)
    img_score = 1.0 / (1.0 + np.exp(-(M @ np.asarray(W_img, f32) + np.asarray(b_img, f32))))
    tl = (tabular @ np.asarray(W_tab, f32) + np.asarray(b_tab, f32)).astype(np.float64)
    tl -= tl.max()
    tab_score = (np.exp(tl) / np.exp(tl).sum()).astype(f32)
    concat = np.concatenate([(img_score * M).astype(f32), (tab_score * tabular).astype(f32)], axis=1)
    logits = (concat @ np.asarray(W_cls, f32) + np.asarray(b_cls, f32)).astype(f32)
    lx = logits.astype(np.float64) - logits.max()
    Y_prob = (np.exp(lx) / np.exp(lx).sum()).astype(f32)
    Y_hat = np.argmax(logits, axis=1).astype(np.int32)
    A_raw = s_full[None, :].astype(f32)
    return logits, Y_prob, Y_hat, A_raw, inst_loss


def kernel(h, tabular, label, W_fc, b_fc, Wa, ba, Wb, bb, Wc, bc,
           W_inst, b_inst, W_img, b_img, W_tab, b_tab, W_cls, b_cls):
    s_full, M_unnorm, _ = _run_device(h, W_fc, Wa, Wb, Wc, b_fc, ba, bb, bc)
    return _finish_host(s_full, M_unnorm, h, tabular, label, W_fc, b_fc,
                        W_inst, b_inst, W_img, b_img, W_tab, b_tab, W_cls, b_cls)
